# revision 41
# baseline (speedup 1.0000x reference)
"""2-layer GraphSAGE (PyG SAGEConv, project=True, mean agg) on 8 trn2 NeuronCores.

Strategy (graph/data parallel, hardcoded for N=50000, E=800000, D=128, 8 cores):
  - Nodes sharded by contiguous ranges of 6250 (padded to 6272 = 49*128) per core.
  - Host preprocesses edges: sorted by (dst core, dst block, src half, src),
    padded so every (block, half) has a uniform chunk count across cores (SPMD);
    x is shipped pre-transposed in fp16 so no on-chip transpose phase is needed.
  - Device per layer:
      * project own rows: p = relu(x @ WpT + bp) -> fp16; two AllGathers (per
        table half, split at per-core row nlh) into [25088,128] fp16 tables so
        lo-half gathers overlap the hi-half collective. Layer-2 projection and
        its AllGathers are interleaved into layer-1's scatter loop; a dummy
        collective at t=0 absorbs the one-time comm-init barrier.
      * gathers: prepare_only SWDGE desc-gen split 4 ways across the 4 SWDGE
        queues (each queue = its own gpsimd core pair -> 4x parallel gen, and
        preps don't wait on table data — only the triggers do). Completion is
        tracked by dedicated per-(queue,half,parity) semaphores with
        total-count waits: order-insensitive, immune to the cross-ring
        completion skew that races tile's cumulative DMASW lanes. Parity
        count == stage bufs so the staging WAR serializes same-sem rounds.
      * scatter via one-hot matmuls: aggT[d,dst] += msg[e,d]^T @ onehot[e,dst],
        one-hots built per block in one DVE is_equal over [128, KT, 128].
      * mean via per-dst invdeg multiply, then output matmuls + bias (+relu).
  - Layer-2 output rows are written per core and concatenated on host.

Perf state (HW-measured): ~1.21ms, rel err 4e-4. Bound by shared-HBM random
256B reads (2 x 28.9MB/core/layer at ~75GB/s effective). Next levers, in
order: fp8 paired-row gathers (halves drain bytes; needs parity-split
one-hots via dloc256=dst+128*(src&1) and 2 matmuls/chunk on oh slices — mind
the DVE broadcast-operand penalty, ~2.4us per [128,KT,128] is_equal), or an
SBUF-resident lo table with src_is_sbuf transpose-mode gathers (needs ~50KB/
partition freed + per-chunk PE transpose). Known dead ends: multi-queue
gathers without dedicated sems (intermittent corruption), >1 group of
untriggered preps (deadlocks the 1024-desc carveout), single_packet=True
(wedges the device), bigger carveout (no gain - drain is bandwidth-bound).
"""

import math
from contextlib import ExitStack

import numpy as np

import concourse.bacc as bacc
import concourse.bass as bass
import concourse.tile as tile
from concourse import library_config, mybir
from concourse.bass_utils import run_bass_kernel_spmd

P = 128
D = 128
CORES = 8
N_NODES = 50000
N_EDGES = 800000

AF = mybir.ActivationFunctionType
OP = mybir.AluOpType
dt = mybir.dt


def _plan(n_nodes, cores):
    nloc = n_nodes // cores
    assert nloc * cores == n_nodes
    nb = math.ceil(nloc / P)
    nloc_pad = nb * P
    npad = cores * nloc_pad
    # lo/hi table halves split each core's rows at nlh so each half can be
    # all-gathered (and consumed) independently: table_lo[c*nlh + r] etc.
    nlh = nloc_pad // 2
    assert cores * nlh < 32768, "dma_gather idx is int16"
    return nloc, nb, nloc_pad, npad, nlh


def preprocess(edge_index, n_nodes, cores):
    """Returns per-core gather/scatter metadata + uniform chunk counts K0, K1."""
    nloc, nb, nloc_pad, npad, nlh = _plan(n_nodes, cores)
    src = np.asarray(edge_index[0], dtype=np.int64)
    dst = np.asarray(edge_index[1], dtype=np.int64)
    E = src.shape[0]

    deg = np.bincount(dst, minlength=n_nodes).astype(np.float64)
    invdeg = (1.0 / np.maximum(deg, 1.0)).astype(np.float32)

    csrc = src // nloc
    lsrc = src - csrc * nloc  # local (padded) row id of source within core
    half = (lsrc >= nlh).astype(np.int64)
    idx_in_half = (csrc * nlh + lsrc - half * nlh).astype(np.int64)

    cdst = dst // nloc
    ldst = dst - cdst * nloc
    blk = ldst // P
    dblk = ldst % P

    # sort edges by (dst core, dst block, src half, src row) — src order gives
    # the DMA engines ascending-address locality within each gather list
    order = np.lexsort((idx_in_half, half, blk, cdst))
    s_half = half[order]
    s_idx = idx_in_half[order]
    s_dblk = dblk[order]
    key = ((cdst[order] * nb + blk[order]) * 2 + s_half).astype(np.int64)

    counts = np.bincount(key, minlength=cores * nb * 2)
    starts = np.zeros(cores * nb * 2 + 1, dtype=np.int64)
    np.cumsum(counts, out=starts[1:])
    rank = np.arange(E, dtype=np.int64) - starts[key]

    cnt = counts.reshape(cores, nb, 2)
    K0 = max(1, int(math.ceil(cnt[:, :, 0].max() / P)))
    K1 = max(1, int(math.ceil(cnt[:, :, 1].max() / P)))

    # idx arrays: [cores, nb, K*P] int16 (pad = 0, harmless row gathered,
    # neutralized by dloc pad = 255 in the one-hot); dloc: [cores, nb, (K0+K1)*P]
    idx0 = np.zeros((cores, nb, K0 * P), dtype=np.int16)
    idx1 = np.zeros((cores, nb, K1 * P), dtype=np.int16)
    dloc = np.full((cores, nb, (K0 + K1) * P), 255.0, dtype=np.float16)

    core_k = key // (nb * 2)
    blk_k = (key // 2) % nb
    m0 = s_half == 0
    m1 = ~m0
    idx0[core_k[m0], blk_k[m0], rank[m0]] = s_idx[m0].astype(np.int16)
    idx1[core_k[m1], blk_k[m1], rank[m1]] = s_idx[m1].astype(np.int16)
    dloc[core_k[m0], blk_k[m0], rank[m0]] = s_dblk[m0].astype(np.float16)
    dloc[core_k[m1], blk_k[m1], K0 * P + rank[m1]] = s_dblk[m1].astype(np.float16)

    def wrap_idx(a):  # [nb, K*P] -> [128, nb*K*P//16] dma_gather layout
        flat = a.reshape(-1)
        w = flat.reshape(-1, 16).T  # [16, I/16]
        return np.tile(w, (8, 1)).copy()

    per_core = []
    for c in range(cores):
        dl = dloc[c].reshape(nb, K0 + K1, P).transpose(2, 0, 1).reshape(P, -1)
        inv = np.ones(nloc_pad, dtype=np.float32)
        inv[:nloc] = invdeg[c * nloc : (c + 1) * nloc]
        per_core.append(
            dict(
                idx0=wrap_idx(idx0[c]),
                idx1=wrap_idx(idx1[c]),
                dloc=np.ascontiguousarray(dl),
                invd=np.broadcast_to(inv[None, :], (P, nloc_pad)).copy(),
            )
        )
    return per_core, K0, K1, invdeg


NQ = 4  # SWDGE queues: each maps to its own gpsimd core pair -> 4x desc-gen
NSPLIT = 4  # sub-gathers per (group, half), round-robined across queues


def _check_lane_queue_alignment(nc):
    """Tile assigns SWDGE completions to 8 DMASW sem lanes round-robin in
    scheduled order; correctness of cross-queue gathers requires each lane to
    only ever carry one hardware queue (per-queue rings are FIFO, lanes count
    cumulatively). Verify the alignment held after scheduling."""
    lane_queues = {}
    for inst in nc.inst_map.values():
        if type(inst).__name__ != "InstDMAGatherAnt":
            continue
        if getattr(inst, "gen_mode", 0) == 1:
            continue  # prepare_only gathers use explicit gsem waits
        si = getattr(inst, "sync_info", None)
        assert si is not None
        for u in si.on_update:
            name = getattr(u, "ant_name", "") or ""
            if name.startswith("DMASW"):
                lane = name.split("_")[0]
                lane_queues.setdefault(lane, set()).add(inst.queue_num)
    for lane, qs in lane_queues.items():
        assert len(qs) == 1, f"sem lane {lane} carries queues {qs}: unsafe"


def build_nc(n_nodes, cores, K0, K1, G, iters=1):
    nloc, nb, nloc_pad, npad, nlh = _plan(n_nodes, cores)
    assert nb % G == 0
    ngroups = nb // G
    KT = K0 + K1
    blo = nlh // P + 1  # proj block count covering the lo half rows

    nc = bacc.Bacc(
        "TRN2",
        target_bir_lowering=False,
        debug=False,
        num_devices=cores,
        num_swdge_queues=NQ,
    )

    xT_d = nc.dram_tensor("xT", [P, nloc_pad], dt.float16, kind="ExternalInput").ap()
    idx0_d = nc.dram_tensor("idx0", [P, nb * K0 * P // 16], dt.int16, kind="ExternalInput").ap()
    idx1_d = nc.dram_tensor("idx1", [P, nb * K1 * P // 16], dt.int16, kind="ExternalInput").ap()
    dloc_d = nc.dram_tensor("dloc", [P, nb * KT], dt.float16, kind="ExternalInput").ap()
    invd_d = nc.dram_tensor("invd", [P, nloc_pad], dt.float32, kind="ExternalInput").ap()
    wdram = {
        n: nc.dram_tensor(n, [P, D], dt.float16, kind="ExternalInput").ap()
        for n in ["Wp1T", "Wl1T", "Wr1T", "Wp2T", "Wl2T", "Wr2T"]
    }
    bp1b_d = nc.dram_tensor("bp1b", [P, D], dt.float32, kind="ExternalInput").ap()
    bl1c_d = nc.dram_tensor("bl1c", [P, 1], dt.float32, kind="ExternalInput").ap()
    bp2b_d = nc.dram_tensor("bp2b", [P, D], dt.float32, kind="ExternalInput").ap()
    bl2b_d = nc.dram_tensor("bl2b", [P, D], dt.float32, kind="ExternalInput").ap()
    iota_d = nc.dram_tensor("iotaw", [P, KT * P], dt.float16, kind="ExternalInput").ap()

    out_own = nc.dram_tensor("out_own", [nloc_pad, D], dt.float32, kind="ExternalOutput").ap()
    h1own = nc.dram_tensor("h1own", [nloc_pad, D], dt.float16).ap()
    h2own = nc.dram_tensor("h2own", [nloc_pad, D], dt.float16).ap()
    t1lo = nc.dram_tensor("t1lo", [cores * nlh, D], dt.float16, addr_space="Shared").ap()
    t1hi = nc.dram_tensor("t1hi", [cores * nlh, D], dt.float16, addr_space="Shared").ap()
    t2lo = nc.dram_tensor("t2lo", [cores * nlh, D], dt.float16, addr_space="Shared").ap()
    t2hi = nc.dram_tensor("t2hi", [cores * nlh, D], dt.float16, addr_space="Shared").ap()
    dumi = nc.dram_tensor("dumi", [16, D], dt.float16).ap()
    dumo = nc.dram_tensor("dumo", [cores * 16, D], dt.float16, addr_space="Shared").ap()

    groups_all = [list(range(cores))]

    with tile.TileContext(nc) as tc, ExitStack() as ctx:
        const = ctx.enter_context(tc.tile_pool(name="const", bufs=1))
        persist = ctx.enter_context(tc.tile_pool(name="persist", bufs=1))
        stage_p = ctx.enter_context(tc.tile_pool(name="stage", bufs=3))
        work = ctx.enter_context(tc.tile_pool(name="work", bufs=3))
        ohp = ctx.enter_context(tc.tile_pool(name="oh", bufs=4))
        aggsb = ctx.enter_context(tc.tile_pool(name="aggsb", bufs=2))
        outp = ctx.enter_context(tc.tile_pool(name="outp", bufs=3))
        psum_agg = ctx.enter_context(tc.tile_pool(name="psum_agg", bufs=4, space="PSUM"))
        psum_mm = ctx.enter_context(tc.tile_pool(name="psum_mm", bufs=2, space="PSUM"))

        nc.gpsimd.load_library(library_config.mlp)

        def cload(ap_dram, shape, dtype, tag):
            t = const.tile(shape, dtype, tag=tag)
            nc.sync.dma_start(t[:], ap_dram)
            return t

        wsb = {n: cload(wdram[n][:, :], [P, D], dt.float16, n) for n in wdram}
        bp1b = cload(bp1b_d[:, :], [P, D], dt.float32, "bp1b")
        bl1c = cload(bl1c_d[:, :], [P, 1], dt.float32, "bl1c")
        bp2b = cload(bp2b_d[:, :], [P, D], dt.float32, "bp2b")
        bl2b = cload(bl2b_d[:, :], [P, D], dt.float32, "bl2b")
        iota_w = cload(iota_d[:, :], [P, KT * P], dt.float16, "iotaw")
        dloc_sb = cload(dloc_d[:, :], [P, nb * KT], dt.float16, "dloc")
        invd_sb = cload(invd_d[:, :], [P, nloc_pad], dt.float32, "invd")
        idx0_sb = cload(idx0_d[:, :], [P, nb * K0 * P // 16], dt.int16, "idx0")
        idx1_sb = cload(idx1_d[:, :], [P, nb * K1 * P // 16], dt.int16, "idx1")

        xT_sb = persist.tile([P, nloc_pad], dt.float16, tag="xT")
        h1T_sb = persist.tile([P, nloc_pad], dt.float16, tag="h1T")

        # Gather-completion semaphores: one per (queue, half, group-parity).
        # sem= on a prepare_only gather is baked into its descriptors; the
        # consumer waits for the round's total (16 incs per sub-gather), an
        # order-insensitive condition. Same-sem rounds are SPAR groups apart
        # and SPAR == staging bufs, so the staging-tile WAR serializes them —
        # cumulative targets can never be satisfied early by a later round.
        SPAR = 3  # must equal stage_p bufs
        gsem = [
            [[nc.alloc_semaphore(f"gs{q}_{h}_{p}") for p in range(SPAR)] for h in range(2)]
            for q in range(NQ)
        ]

        def _iter_body():
            for qs in gsem:
                for hs in qs:
                    for s in hs:
                        nc.gpsimd.sem_clear(s)
            gtgt = [[[0] * SPAR for _ in range(2)] for _ in range(NQ)]
            grp = [0]  # global group counter: parity follows pool rotation

            # dummy collective: absorbs the one-time comm-init barrier while
            # the projection phase runs
            nc.gpsimd.collective_compute(
                "AllGather", OP.bypass, replica_groups=groups_all,
                ins=[dumi[:, :]], outs=[dumo[:, :]],
            )

            def proj2_block(b):
                sl = slice(b * P, (b + 1) * P)
                p_ps = psum_mm.tile([P, D], dt.float32, tag="mm")
                nc.tensor.matmul(p_ps[:], lhsT=h1T_sb[:, sl], rhs=wsb["Wp2T"][:], start=True, stop=True)
                pb = work.tile([P, D], dt.float32, tag="pb")
                nc.vector.tensor_tensor(out=pb[:], in0=p_ps[:], in1=bp2b[:], op=OP.add)
                pr = outp.tile([P, D], dt.float16, tag="pr")
                nc.scalar.activation(pr[:], pb[:], AF.Relu)
                nc.sync.dma_start(h2own[sl, :], pr[:])

            def ag(h_own, tlo, thi, part):
                nc.gpsimd.collective_compute(
                    "AllGather", OP.bypass, replica_groups=groups_all,
                    ins=[h_own[0:nlh, :] if part == 0 else h_own[nlh:nloc_pad, :]],
                    outs=[(tlo if part == 0 else thi)[:, :]],
                )

            # ---------------- Phase A: layer-1 projection of own rows ----------
            nc.sync.dma_start(xT_sb[:], xT_d[:, :])
            for b in range(nb):
                sl = slice(b * P, (b + 1) * P)
                p_ps = psum_mm.tile([P, D], dt.float32, tag="mm")
                nc.tensor.matmul(p_ps[:], lhsT=xT_sb[:, sl], rhs=wsb["Wp1T"][:], start=True, stop=True)
                pb = work.tile([P, D], dt.float32, tag="pb")
                nc.vector.tensor_tensor(out=pb[:], in0=p_ps[:], in1=bp1b[:], op=OP.add)
                pr = outp.tile([P, D], dt.float16, tag="pr")
                nc.scalar.activation(pr[:], pb[:], AF.Relu)
                nc.sync.dma_start(h1own[sl, :], pr[:])
                if b == blo - 1:
                    ag(h1own, t1lo, t1hi, 0)
            ag(h1own, t1lo, t1hi, 1)

            def split_ranges(n):
                step = math.ceil(n / NSPLIT)
                return [(s, min(s + step, n)) for s in range(0, n, step)]

            # ---------------- message+aggregate for one layer -------------------
            # prepare_only sub-gathers fan descriptor generation across the 4
            # SWDGE queue core-pairs and don't wait for table data (only the
            # triggers carry that dep, so desc-gen hides under the AllGather).
            # One prep per queue per trigger round keeps the untriggered ring
            # footprint at the same level as a regular triggered gather.
            def agg_layer(tlo, thi, root_sb, WlT, WrT, layer):
                staged = {}

                def prep_group(g):
                    par = grp[0] % SPAR
                    grp[0] += 1
                    st0 = stage_p.tile([P, G * K0, D], dt.float16, tag="st0")
                    st1 = stage_p.tile([P, G * K1, D], dt.float16, tag="st1")
                    staged[g] = (st0, st1, par)
                    for h, (st, idx_sb, K) in enumerate((
                        (st0, idx0_sb, K0),
                        (st1, idx1_sb, K1),
                    )):
                        tab = tlo if h == 0 else thi
                        gcols = g * G * K * P // 16
                        for j, (s, e) in enumerate(split_ranges(G * K)):
                            q = j % NQ
                            nc.gpsimd.dma_gather(
                                st[:, s:e, :],
                                tab[:, :],
                                idx_sb[:, gcols + s * 8 : gcols + e * 8],
                                (e - s) * P, (e - s) * P, D,
                                single_packet=False,
                                queue_num=q,
                                prepare_only=True,
                                sem=gsem[q][h][par],
                            )
                            gtgt[q][h][par] += 16
                        for q in range(NQ):
                            nc.gpsimd.trigger_dma(count=None, queue_num=q)

                for g in range(ngroups):
                    prep_group(g)
                    st0, st1, par = staged.pop(g)
                    first = True
                    for bb in range(G):
                        b = g * G + bb
                        sl = slice(b * P, (b + 1) * P)
                        ohb = ohp.tile([P, KT, P], dt.float16)
                        nc.vector.tensor_tensor(
                            out=ohb[:],
                            in0=dloc_sb[:, b * KT : (b + 1) * KT].to_broadcast([P, KT, P]),
                            in1=iota_w[:, :].rearrange("p (k d) -> p k d", k=KT),
                            op=OP.is_equal,
                        )
                        if first:
                            for q in range(NQ):
                                for h in range(2):
                                    nc.tensor.wait_ge(gsem[q][h][par], gtgt[q][h][par])
                            first = False
                        agg_ps = psum_agg.tile([P, P], dt.float32)
                        for t in range(KT):
                            msg = st0[:, bb * K0 + t, :] if t < K0 else st1[:, bb * K1 + (t - K0), :]
                            nc.tensor.matmul(
                                agg_ps[:], lhsT=msg, rhs=ohb[:, t, :],
                                start=(t == 0), stop=(t == KT - 1),
                            )
                        aggT = aggsb.tile([P, P], dt.float16)
                        nc.vector.tensor_tensor(
                            out=aggT[:], in0=agg_ps[:], in1=invd_sb[:, sl], op=OP.mult
                        )
                        if layer == 1:
                            o_ps = psum_mm.tile([P, P], dt.float32, tag="mm")
                            nc.tensor.matmul(o_ps[:], lhsT=WlT[:], rhs=aggT[:], start=True, stop=False)
                            nc.tensor.matmul(o_ps[:], lhsT=WrT[:], rhs=root_sb[:, sl], start=False, stop=True)
                            nc.scalar.activation(h1T_sb[:, sl], o_ps[:], AF.Relu, bias=bl1c[:], scale=1.0)
                            proj2_block(b)
                            if b == blo - 1:
                                ag(h2own, t2lo, t2hi, 0)
                            elif b == nb - 1:
                                ag(h2own, t2lo, t2hi, 1)
                        else:
                            o_ps = psum_mm.tile([P, D], dt.float32, tag="mm")
                            nc.tensor.matmul(o_ps[:], lhsT=aggT[:], rhs=WlT[:], start=True, stop=False)
                            nc.tensor.matmul(o_ps[:], lhsT=root_sb[:, sl], rhs=WrT[:], start=False, stop=True)
                            ob = outp.tile([P, D], dt.float32, tag="ob")
                            nc.vector.tensor_tensor(out=ob[:], in0=o_ps[:], in1=bl2b[:], op=OP.add)
                            nc.sync.dma_start(out_own[sl, :], ob[:])

            # ---------------- Phase B: layer-1 aggregate (+proj2, AG2) ---------
            agg_layer(t1lo, t1hi, xT_sb, wsb["Wl1T"], wsb["Wr1T"], layer=1)

            # ---------------- Phase D: layer-2 aggregate -> out ----------------
            agg_layer(t2lo, t2hi, h1T_sb, wsb["Wl2T"], wsb["Wr2T"], layer=2)

        for _ in range(iters):
            _iter_body()

    nc.compile()
    _check_lane_queue_alignment(nc)
    return nc


def make_in_maps(inputs, per_core, n_nodes, cores, KT):
    nloc, nb, nloc_pad, npad, nhalf = _plan(n_nodes, cores)
    x = np.asarray(inputs["x"], dtype=np.float32)
    consts = dict(
        Wp1T=np.asarray(inputs["Wp1"]).T.astype(np.float16),
        Wl1T=np.asarray(inputs["Wl1"]).T.astype(np.float16),
        Wr1T=np.asarray(inputs["Wr1"]).T.astype(np.float16),
        Wp2T=np.asarray(inputs["Wp2"]).T.astype(np.float16),
        Wl2T=np.asarray(inputs["Wl2"]).T.astype(np.float16),
        Wr2T=np.asarray(inputs["Wr2"]).T.astype(np.float16),
        bp1b=np.broadcast_to(np.asarray(inputs["bp1"], np.float32)[None, :], (P, D)).copy(),
        bl1c=np.asarray(inputs["bl1"], np.float32).reshape(P, 1).copy(),
        bp2b=np.broadcast_to(np.asarray(inputs["bp2"], np.float32)[None, :], (P, D)).copy(),
        bl2b=np.broadcast_to(np.asarray(inputs["bl2"], np.float32)[None, :], (P, D)).copy(),
        iotaw=np.broadcast_to(
            np.tile(np.arange(P, dtype=np.float16), KT)[None, :], (P, KT * P)
        ).copy(),
    )
    in_maps = []
    for c in range(cores):
        xo = np.zeros((nloc_pad, D), dtype=np.float32)
        xo[:nloc] = x[c * nloc : (c + 1) * nloc]
        m = dict(consts)
        m["xT"] = np.ascontiguousarray(xo.T.astype(np.float16))
        m.update(per_core[c])
        in_maps.append(m)
    return in_maps


_BUILT = {}


def _run(inputs, n_nodes, n_edges, cores, G, trace=False):
    per_core, K0, K1, _ = preprocess(inputs["edge_index"], n_nodes, cores)
    key = (n_nodes, cores, K0, K1, G)
    if key not in _BUILT:
        _BUILT[key] = build_nc(n_nodes, cores, K0, K1, G)
    nc = _BUILT[key]
    in_maps = make_in_maps(inputs, per_core, n_nodes, cores, K0 + K1)
    res = run_bass_kernel_spmd(nc, in_maps, list(range(cores)), trace=trace)
    nloc, nb, nloc_pad, npad, nhalf = _plan(n_nodes, cores)
    out = np.concatenate([res.results[c]["out_own"][:nloc] for c in range(cores)], axis=0)
    return out.astype(np.float32), res


def kernel(**inputs):
    out, _ = _run(inputs, N_NODES, N_EDGES, CORES, G=7)
    return out



# revision 44
# speedup vs baseline: 1.0087x; 1.0087x over previous
"""2-layer GraphSAGE (PyG SAGEConv, project=True, mean agg) on 8 trn2 NeuronCores.

Strategy (graph/data parallel, hardcoded for N=50000, E=800000, D=128, 8 cores):
  - Nodes sharded by contiguous ranges of 6250 (padded to 6272 = 49*128) per core.
  - Host preprocesses edges: sorted by (dst core, dst block, src half, src),
    padded so every (block, half) has a uniform chunk count across cores (SPMD);
    x is shipped pre-transposed in fp16 so no on-chip transpose phase is needed.
  - Device per layer:
      * project own rows: p = relu(x @ WpT + bp) -> fp16; two AllGathers (per
        table half, split at per-core row nlh) into [25088,128] fp16 tables so
        lo-half gathers overlap the hi-half collective. Layer-2 projection and
        its AllGathers are interleaved into layer-1's scatter loop; a dummy
        collective at t=0 absorbs the one-time comm-init barrier.
      * gathers: prepare_only SWDGE desc-gen split 4 ways across the 4 SWDGE
        queues (each queue = its own gpsimd core pair -> 4x parallel gen, and
        preps don't wait on table data — only the triggers do). Completion is
        tracked by dedicated per-(queue,half,parity) semaphores with
        total-count waits: order-insensitive, immune to the cross-ring
        completion skew that races tile's cumulative DMASW lanes. Parity
        count == stage bufs so the staging WAR serializes same-sem rounds.
      * scatter via one-hot matmuls: aggT[d,dst] += msg[e,d]^T @ onehot[e,dst],
        one-hots built per block in one DVE is_equal over [128, KT, 128].
      * mean via per-dst invdeg multiply, then output matmuls + bias (+relu).
  - Layer-2 output rows are written per core and concatenated on host.

Perf state (HW-measured): ~1.21ms, rel err 4e-4. Bound by shared-HBM random
256B reads (2 x 28.9MB/core/layer at ~75GB/s effective). Next levers, in
order: fp8 paired-row gathers (halves drain bytes; needs parity-split
one-hots via dloc256=dst+128*(src&1) and 2 matmuls/chunk on oh slices — mind
the DVE broadcast-operand penalty, ~2.4us per [128,KT,128] is_equal), or an
SBUF-resident lo table with src_is_sbuf transpose-mode gathers (needs ~50KB/
partition freed + per-chunk PE transpose). Known dead ends: multi-queue
gathers without dedicated sems (intermittent corruption), >1 group of
untriggered preps (deadlocks the 1024-desc carveout), single_packet=True
(wedges the device), bigger carveout (no gain - drain is bandwidth-bound).
"""

import math
from contextlib import ExitStack

import numpy as np

import concourse.bacc as bacc
import concourse.bass as bass
import concourse.tile as tile
from concourse import library_config, mybir
from concourse.bass_utils import run_bass_kernel_spmd

P = 128
D = 128
CORES = 8
N_NODES = 50000
N_EDGES = 800000

AF = mybir.ActivationFunctionType
OP = mybir.AluOpType
dt = mybir.dt


def _plan(n_nodes, cores):
    nloc = n_nodes // cores
    assert nloc * cores == n_nodes
    nb = math.ceil(nloc / P)
    nloc_pad = nb * P
    npad = cores * nloc_pad
    # lo/hi table halves split each core's rows at nlh so each half can be
    # all-gathered (and consumed) independently: table_lo[c*nlh + r] etc.
    nlh = nloc_pad // 2
    assert cores * nlh < 32768, "dma_gather idx is int16"
    return nloc, nb, nloc_pad, npad, nlh


def preprocess(edge_index, n_nodes, cores):
    """Returns per-core gather/scatter metadata + uniform chunk counts K0, K1."""
    nloc, nb, nloc_pad, npad, nlh = _plan(n_nodes, cores)
    src = np.asarray(edge_index[0], dtype=np.int64)
    dst = np.asarray(edge_index[1], dtype=np.int64)
    E = src.shape[0]

    deg = np.bincount(dst, minlength=n_nodes).astype(np.float64)
    invdeg = (1.0 / np.maximum(deg, 1.0)).astype(np.float32)

    csrc = src // nloc
    lsrc = src - csrc * nloc  # local (padded) row id of source within core
    half = (lsrc >= nlh).astype(np.int64)
    idx_in_half = (csrc * nlh + lsrc - half * nlh).astype(np.int64)

    cdst = dst // nloc
    ldst = dst - cdst * nloc
    blk = ldst // P
    dblk = ldst % P

    # sort edges by (dst core, dst block, src half, src row) — src order gives
    # the DMA engines ascending-address locality within each gather list
    order = np.lexsort((idx_in_half, half, blk, cdst))
    s_half = half[order]
    s_idx = idx_in_half[order]
    s_dblk = dblk[order]
    key = ((cdst[order] * nb + blk[order]) * 2 + s_half).astype(np.int64)

    counts = np.bincount(key, minlength=cores * nb * 2)
    starts = np.zeros(cores * nb * 2 + 1, dtype=np.int64)
    np.cumsum(counts, out=starts[1:])
    rank = np.arange(E, dtype=np.int64) - starts[key]

    cnt = counts.reshape(cores, nb, 2)
    K0 = max(1, int(math.ceil(cnt[:, :, 0].max() / P)))
    K1 = max(1, int(math.ceil(cnt[:, :, 1].max() / P)))

    # idx arrays: [cores, nb, K*P] int16 (pad = 0, harmless row gathered,
    # neutralized by dloc pad = 255 in the one-hot); dloc: [cores, nb, (K0+K1)*P]
    idx0 = np.zeros((cores, nb, K0 * P), dtype=np.int16)
    idx1 = np.zeros((cores, nb, K1 * P), dtype=np.int16)
    dloc = np.full((cores, nb, (K0 + K1) * P), 255.0, dtype=np.float16)

    core_k = key // (nb * 2)
    blk_k = (key // 2) % nb
    m0 = s_half == 0
    m1 = ~m0
    idx0[core_k[m0], blk_k[m0], rank[m0]] = s_idx[m0].astype(np.int16)
    idx1[core_k[m1], blk_k[m1], rank[m1]] = s_idx[m1].astype(np.int16)
    dloc[core_k[m0], blk_k[m0], rank[m0]] = s_dblk[m0].astype(np.float16)
    dloc[core_k[m1], blk_k[m1], K0 * P + rank[m1]] = s_dblk[m1].astype(np.float16)

    def wrap_idx(a):  # [nb, K*P] -> [128, nb*K*P//16] dma_gather layout
        flat = a.reshape(-1)
        w = flat.reshape(-1, 16).T  # [16, I/16]
        return np.tile(w, (8, 1)).copy()

    per_core = []
    for c in range(cores):
        dl = dloc[c].reshape(nb, K0 + K1, P).transpose(2, 0, 1).reshape(P, -1)
        inv = np.ones(nloc_pad, dtype=np.float32)
        inv[:nloc] = invdeg[c * nloc : (c + 1) * nloc]
        per_core.append(
            dict(
                idx0=wrap_idx(idx0[c]),
                idx1=wrap_idx(idx1[c]),
                dloc=np.ascontiguousarray(dl),
                invd=np.broadcast_to(inv[None, :], (P, nloc_pad)).copy(),
            )
        )
    return per_core, K0, K1, invdeg


NQ = 4  # SWDGE queues: each maps to its own gpsimd core pair -> 4x desc-gen
NSPLIT = 4  # sub-gathers per (group, half), round-robined across queues


def _check_lane_queue_alignment(nc):
    """Tile assigns SWDGE completions to 8 DMASW sem lanes round-robin in
    scheduled order; correctness of cross-queue gathers requires each lane to
    only ever carry one hardware queue (per-queue rings are FIFO, lanes count
    cumulatively). Verify the alignment held after scheduling."""
    lane_queues = {}
    for inst in nc.inst_map.values():
        if type(inst).__name__ != "InstDMAGatherAnt":
            continue
        if getattr(inst, "gen_mode", 0) == 1:
            continue  # prepare_only gathers use explicit gsem waits
        si = getattr(inst, "sync_info", None)
        assert si is not None
        for u in si.on_update:
            name = getattr(u, "ant_name", "") or ""
            if name.startswith("DMASW"):
                lane = name.split("_")[0]
                lane_queues.setdefault(lane, set()).add(inst.queue_num)
    for lane, qs in lane_queues.items():
        assert len(qs) == 1, f"sem lane {lane} carries queues {qs}: unsafe"


def build_nc(n_nodes, cores, K0, K1, G, iters=1):
    nloc, nb, nloc_pad, npad, nlh = _plan(n_nodes, cores)
    assert nb % G == 0
    ngroups = nb // G
    KT = K0 + K1
    blo = nlh // P + 1  # proj block count covering the lo half rows

    nc = bacc.Bacc(
        "TRN2",
        target_bir_lowering=False,
        debug=False,
        num_devices=cores,
        num_swdge_queues=NQ,
    )

    xT_d = nc.dram_tensor("xT", [P, nloc_pad], dt.float16, kind="ExternalInput").ap()
    idx0_d = nc.dram_tensor("idx0", [P, nb * K0 * P // 16], dt.int16, kind="ExternalInput").ap()
    idx1_d = nc.dram_tensor("idx1", [P, nb * K1 * P // 16], dt.int16, kind="ExternalInput").ap()
    dloc_d = nc.dram_tensor("dloc", [P, nb * KT], dt.float16, kind="ExternalInput").ap()
    invd_d = nc.dram_tensor("invd", [P, nloc_pad], dt.float32, kind="ExternalInput").ap()
    wdram = {
        n: nc.dram_tensor(n, [P, D], dt.float16, kind="ExternalInput").ap()
        for n in ["Wp1T", "Wl1T", "Wr1T", "Wp2T", "Wl2T", "Wr2T"]
    }
    bp1b_d = nc.dram_tensor("bp1b", [P, D], dt.float32, kind="ExternalInput").ap()
    bl1c_d = nc.dram_tensor("bl1c", [P, 1], dt.float32, kind="ExternalInput").ap()
    bp2b_d = nc.dram_tensor("bp2b", [P, D], dt.float32, kind="ExternalInput").ap()
    bl2b_d = nc.dram_tensor("bl2b", [P, D], dt.float32, kind="ExternalInput").ap()
    iota_d = nc.dram_tensor("iotaw", [P, KT * P], dt.float16, kind="ExternalInput").ap()

    out_own = nc.dram_tensor("out_own", [nloc_pad, D], dt.float32, kind="ExternalOutput").ap()
    h1own = nc.dram_tensor("h1own", [nloc_pad, D], dt.float16).ap()
    h2own = nc.dram_tensor("h2own", [nloc_pad, D], dt.float16).ap()
    t1lo = nc.dram_tensor("t1lo", [cores * nlh, D], dt.float16, addr_space="Shared").ap()
    t1hi = nc.dram_tensor("t1hi", [cores * nlh, D], dt.float16, addr_space="Shared").ap()
    t2lo = nc.dram_tensor("t2lo", [cores * nlh, D], dt.float16, addr_space="Shared").ap()
    t2hi = nc.dram_tensor("t2hi", [cores * nlh, D], dt.float16, addr_space="Shared").ap()
    dumi = nc.dram_tensor("dumi", [16, D], dt.float16).ap()
    dumo = nc.dram_tensor("dumo", [cores * 16, D], dt.float16, addr_space="Shared").ap()

    groups_all = [list(range(cores))]

    with tile.TileContext(nc) as tc, ExitStack() as ctx:
        const = ctx.enter_context(tc.tile_pool(name="const", bufs=1))
        persist = ctx.enter_context(tc.tile_pool(name="persist", bufs=1))
        stage_p = ctx.enter_context(tc.tile_pool(name="stage", bufs=3))
        work = ctx.enter_context(tc.tile_pool(name="work", bufs=3))
        ohp = ctx.enter_context(tc.tile_pool(name="oh", bufs=4))
        aggsb = ctx.enter_context(tc.tile_pool(name="aggsb", bufs=2))
        outp = ctx.enter_context(tc.tile_pool(name="outp", bufs=3))
        psum_agg = ctx.enter_context(tc.tile_pool(name="psum_agg", bufs=4, space="PSUM"))
        psum_mm = ctx.enter_context(tc.tile_pool(name="psum_mm", bufs=2, space="PSUM"))

        nc.gpsimd.load_library(library_config.mlp)

        def cload(ap_dram, shape, dtype, tag):
            t = const.tile(shape, dtype, tag=tag)
            nc.sync.dma_start(t[:], ap_dram)
            return t

        wsb = {n: cload(wdram[n][:, :], [P, D], dt.float16, n) for n in wdram}
        bp1b = cload(bp1b_d[:, :], [P, D], dt.float32, "bp1b")
        bl1c = cload(bl1c_d[:, :], [P, 1], dt.float32, "bl1c")
        bp2b = cload(bp2b_d[:, :], [P, D], dt.float32, "bp2b")
        bl2b = cload(bl2b_d[:, :], [P, D], dt.float32, "bl2b")
        iota_w = cload(iota_d[:, :], [P, KT * P], dt.float16, "iotaw")
        dloc_sb = cload(dloc_d[:, :], [P, nb * KT], dt.float16, "dloc")
        invd_sb = cload(invd_d[:, :], [P, nloc_pad], dt.float32, "invd")
        idx0_sb = cload(idx0_d[:, :], [P, nb * K0 * P // 16], dt.int16, "idx0")
        idx1_sb = cload(idx1_d[:, :], [P, nb * K1 * P // 16], dt.int16, "idx1")

        xT_sb = persist.tile([P, nloc_pad], dt.float16, tag="xT")
        h1T_sb = persist.tile([P, nloc_pad], dt.float16, tag="h1T")

        # Gather-completion semaphores: one per (queue, half, group-parity).
        # sem= on a prepare_only gather is baked into its descriptors; the
        # consumer waits for the round's total (16 incs per sub-gather), an
        # order-insensitive condition. Same-sem rounds are SPAR groups apart
        # and SPAR == staging bufs, so the staging-tile WAR serializes them —
        # cumulative targets can never be satisfied early by a later round.
        SPAR = 3  # must equal stage_p bufs
        gsem = [
            [[nc.alloc_semaphore(f"gs{q}_{h}_{p}") for p in range(SPAR)] for h in range(2)]
            for q in range(NQ)
        ]

        def _iter_body():
            for qs in gsem:
                for hs in qs:
                    for s in hs:
                        nc.gpsimd.sem_clear(s)
            gtgt = [[[0] * SPAR for _ in range(2)] for _ in range(NQ)]
            grp = [0]  # global group counter: parity follows pool rotation

            # dummy collective: absorbs the one-time comm-init barrier while
            # the projection phase runs
            nc.gpsimd.collective_compute(
                "AllGather", OP.bypass, replica_groups=groups_all,
                ins=[dumi[:, :]], outs=[dumo[:, :]],
            )

            def proj2_block(b):
                sl = slice(b * P, (b + 1) * P)
                p_ps = psum_mm.tile([P, D], dt.float32, tag="mm")
                nc.tensor.matmul(p_ps[:], lhsT=h1T_sb[:, sl], rhs=wsb["Wp2T"][:], start=True, stop=True)
                pb = work.tile([P, D], dt.float32, tag="pb")
                nc.vector.tensor_tensor(out=pb[:], in0=p_ps[:], in1=bp2b[:], op=OP.add)
                pr = outp.tile([P, D], dt.float16, tag="pr")
                nc.scalar.activation(pr[:], pb[:], AF.Relu)
                nc.sync.dma_start(h2own[sl, :], pr[:])

            def ag(h_own, tlo, thi, part):
                nc.gpsimd.collective_compute(
                    "AllGather", OP.bypass, replica_groups=groups_all,
                    ins=[h_own[0:nlh, :] if part == 0 else h_own[nlh:nloc_pad, :]],
                    outs=[(tlo if part == 0 else thi)[:, :]],
                )

            def split_ranges(n):
                step = math.ceil(n / NSPLIT)
                return [(s, min(s + step, n)) for s in range(0, n, step)]

            # ---------------- message+aggregate for one layer -------------------
            # prepare_only sub-gathers fan descriptor generation across the 4
            # SWDGE queue core-pairs and don't wait for table data (only the
            # triggers carry that dep, so desc-gen hides under the AllGather).
            # One prep per queue per trigger round keeps the untriggered ring
            # footprint at the same level as a regular triggered gather.
            def prep_group(staged, g, tlo, thi, trigger=True):
                par = grp[0] % SPAR
                grp[0] += 1
                st0 = stage_p.tile([P, G * K0, D], dt.float16, tag="st0")
                st1 = stage_p.tile([P, G * K1, D], dt.float16, tag="st1")
                staged[g] = (st0, st1, par)
                for h, (st, idx_sb, K) in enumerate((
                    (st0, idx0_sb, K0),
                    (st1, idx1_sb, K1),
                )):
                    tab = tlo if h == 0 else thi
                    gcols = g * G * K * P // 16
                    for j, (s, e) in enumerate(split_ranges(G * K)):
                        q = j % NQ
                        nc.gpsimd.dma_gather(
                            st[:, s:e, :],
                            tab[:, :],
                            idx_sb[:, gcols + s * 8 : gcols + e * 8],
                            (e - s) * P, (e - s) * P, D,
                            single_packet=False,
                            queue_num=q,
                            prepare_only=True,
                            sem=gsem[q][h][par],
                        )
                        gtgt[q][h][par] += 16
                    if trigger:
                        for q in range(NQ):
                            nc.gpsimd.trigger_dma(count=None, queue_num=q)

            # ---------------- Phase A: layer-1 projection of own rows ----------
            nc.sync.dma_start(xT_sb[:], xT_d[:, :])
            for b in range(nb):
                sl = slice(b * P, (b + 1) * P)
                p_ps = psum_mm.tile([P, D], dt.float32, tag="mm")
                nc.tensor.matmul(p_ps[:], lhsT=xT_sb[:, sl], rhs=wsb["Wp1T"][:], start=True, stop=True)
                pb = work.tile([P, D], dt.float32, tag="pb")
                nc.vector.tensor_tensor(out=pb[:], in0=p_ps[:], in1=bp1b[:], op=OP.add)
                pr = outp.tile([P, D], dt.float16, tag="pr")
                nc.scalar.activation(pr[:], pb[:], AF.Relu)
                nc.sync.dma_start(h1own[sl, :], pr[:])
                if b == blo - 1:
                    ag(h1own, t1lo, t1hi, 0)
            ag(h1own, t1lo, t1hi, 1)

            def agg_layer(tlo, thi, root_sb, WlT, WrT, layer, staged):
                for g in range(ngroups):
                    if g not in staged:
                        prep_group(staged, g, tlo, thi)
                    else:
                        # pre-prepped before Phase A: fire its deferred triggers
                        for q in range(NQ):
                            nc.gpsimd.trigger_dma(count=None, queue_num=q)
                    st0, st1, par = staged.pop(g)
                    first = True
                    for bb in range(G):
                        b = g * G + bb
                        sl = slice(b * P, (b + 1) * P)
                        ohb = ohp.tile([P, KT, P], dt.float16)
                        nc.vector.tensor_tensor(
                            out=ohb[:],
                            in0=dloc_sb[:, b * KT : (b + 1) * KT].to_broadcast([P, KT, P]),
                            in1=iota_w[:, :].rearrange("p (k d) -> p k d", k=KT),
                            op=OP.is_equal,
                        )
                        if first:
                            for q in range(NQ):
                                for h in range(2):
                                    nc.tensor.wait_ge(gsem[q][h][par], gtgt[q][h][par])
                            first = False
                        agg_ps = psum_agg.tile([P, P], dt.float32)
                        for t in range(KT):
                            msg = st0[:, bb * K0 + t, :] if t < K0 else st1[:, bb * K1 + (t - K0), :]
                            nc.tensor.matmul(
                                agg_ps[:], lhsT=msg, rhs=ohb[:, t, :],
                                start=(t == 0), stop=(t == KT - 1),
                            )
                        aggT = aggsb.tile([P, P], dt.float16)
                        nc.vector.tensor_tensor(
                            out=aggT[:], in0=agg_ps[:], in1=invd_sb[:, sl], op=OP.mult
                        )
                        if layer == 1:
                            o_ps = psum_mm.tile([P, P], dt.float32, tag="mm")
                            nc.tensor.matmul(o_ps[:], lhsT=WlT[:], rhs=aggT[:], start=True, stop=False)
                            nc.tensor.matmul(o_ps[:], lhsT=WrT[:], rhs=root_sb[:, sl], start=False, stop=True)
                            nc.scalar.activation(h1T_sb[:, sl], o_ps[:], AF.Relu, bias=bl1c[:], scale=1.0)
                            proj2_block(b)
                            if b == blo - 1:
                                ag(h2own, t2lo, t2hi, 0)
                            elif b == nb - 1:
                                ag(h2own, t2lo, t2hi, 1)
                        else:
                            o_ps = psum_mm.tile([P, D], dt.float32, tag="mm")
                            nc.tensor.matmul(o_ps[:], lhsT=aggT[:], rhs=WlT[:], start=True, stop=False)
                            nc.tensor.matmul(o_ps[:], lhsT=root_sb[:, sl], rhs=WrT[:], start=False, stop=True)
                            ob = outp.tile([P, D], dt.float32, tag="ob")
                            nc.vector.tensor_tensor(out=ob[:], in0=o_ps[:], in1=bl2b[:], op=OP.add)
                            nc.sync.dma_start(out_own[sl, :], ob[:])

            # ---------------- Phase B: layer-1 aggregate (+proj2, AG2) ---------
            agg_layer(t1lo, t1hi, xT_sb, wsb["Wl1T"], wsb["Wr1T"], layer=1, staged={})

            # ---------------- Phase D: layer-2 aggregate -> out ----------------
            agg_layer(t2lo, t2hi, h1T_sb, wsb["Wl2T"], wsb["Wr2T"], layer=2, staged={})

        for _ in range(iters):
            _iter_body()

    nc.compile()
    _check_lane_queue_alignment(nc)
    return nc


def make_in_maps(inputs, per_core, n_nodes, cores, KT):
    nloc, nb, nloc_pad, npad, nhalf = _plan(n_nodes, cores)
    x = np.asarray(inputs["x"], dtype=np.float32)
    consts = dict(
        Wp1T=np.asarray(inputs["Wp1"]).T.astype(np.float16),
        Wl1T=np.asarray(inputs["Wl1"]).T.astype(np.float16),
        Wr1T=np.asarray(inputs["Wr1"]).T.astype(np.float16),
        Wp2T=np.asarray(inputs["Wp2"]).T.astype(np.float16),
        Wl2T=np.asarray(inputs["Wl2"]).T.astype(np.float16),
        Wr2T=np.asarray(inputs["Wr2"]).T.astype(np.float16),
        bp1b=np.broadcast_to(np.asarray(inputs["bp1"], np.float32)[None, :], (P, D)).copy(),
        bl1c=np.asarray(inputs["bl1"], np.float32).reshape(P, 1).copy(),
        bp2b=np.broadcast_to(np.asarray(inputs["bp2"], np.float32)[None, :], (P, D)).copy(),
        bl2b=np.broadcast_to(np.asarray(inputs["bl2"], np.float32)[None, :], (P, D)).copy(),
        iotaw=np.broadcast_to(
            np.tile(np.arange(P, dtype=np.float16), KT)[None, :], (P, KT * P)
        ).copy(),
    )
    in_maps = []
    for c in range(cores):
        xo = np.zeros((nloc_pad, D), dtype=np.float32)
        xo[:nloc] = x[c * nloc : (c + 1) * nloc]
        m = dict(consts)
        m["xT"] = np.ascontiguousarray(xo.T.astype(np.float16))
        m.update(per_core[c])
        in_maps.append(m)
    return in_maps


_BUILT = {}


def _run(inputs, n_nodes, n_edges, cores, G, trace=False):
    per_core, K0, K1, _ = preprocess(inputs["edge_index"], n_nodes, cores)
    key = (n_nodes, cores, K0, K1, G)
    if key not in _BUILT:
        _BUILT[key] = build_nc(n_nodes, cores, K0, K1, G)
    nc = _BUILT[key]
    in_maps = make_in_maps(inputs, per_core, n_nodes, cores, K0 + K1)
    res = run_bass_kernel_spmd(nc, in_maps, list(range(cores)), trace=trace)
    nloc, nb, nloc_pad, npad, nhalf = _plan(n_nodes, cores)
    out = np.concatenate([res.results[c]["out_own"][:nloc] for c in range(cores)], axis=0)
    return out.astype(np.float32), res


def kernel(**inputs):
    out, _ = _run(inputs, N_NODES, N_EDGES, CORES, G=7)
    return out



# revision 45
# speedup vs baseline: 1.0935x; 1.0841x over previous
"""2-layer GraphSAGE (PyG SAGEConv, project=True, mean agg) on 8 trn2 NeuronCores.

Strategy (graph/data parallel, hardcoded for N=50000, E=800000, D=128, 8 cores):
  - Nodes sharded by contiguous ranges of 6250 (padded to 6272 = 49*128) per core.
  - Host preprocesses edges: sorted by (dst core, dst block, src half, src),
    padded so every (block, half) has a uniform chunk count across cores (SPMD);
    x is shipped pre-transposed in fp16 so no on-chip transpose phase is needed.
  - Device per layer:
      * project own rows: p = relu(x @ WpT + bp) -> fp16; two AllGathers (per
        table half, split at per-core row nlh) into [25088,128] fp16 tables so
        lo-half gathers overlap the hi-half collective. Layer-2 projection and
        its AllGathers are interleaved into layer-1's scatter loop; a dummy
        collective at t=0 absorbs the one-time comm-init barrier.
      * gathers: prepare_only SWDGE desc-gen split 4 ways across the 4 SWDGE
        queues (each queue = its own gpsimd core pair -> 4x parallel gen, and
        preps don't wait on table data — only the triggers do). Completion is
        tracked by dedicated per-(queue,half,parity) semaphores with
        total-count waits: order-insensitive, immune to the cross-ring
        completion skew that races tile's cumulative DMASW lanes. Parity
        count == stage bufs so the staging WAR serializes same-sem rounds.
      * scatter via one-hot matmuls: aggT[d,dst] += msg[e,d]^T @ onehot[e,dst],
        one-hots built per block in one DVE is_equal over [128, KT, 128].
      * mean via per-dst invdeg multiply, then output matmuls + bias (+relu).
  - Layer-2 output rows are written per core and concatenated on host.

Perf state (HW-measured): ~1.21ms, rel err 4e-4. Bound by shared-HBM random
256B reads (2 x 28.9MB/core/layer at ~75GB/s effective). Next levers, in
order: fp8 paired-row gathers (halves drain bytes; needs parity-split
one-hots via dloc256=dst+128*(src&1) and 2 matmuls/chunk on oh slices — mind
the DVE broadcast-operand penalty, ~2.4us per [128,KT,128] is_equal), or an
SBUF-resident lo table with src_is_sbuf transpose-mode gathers (needs ~50KB/
partition freed + per-chunk PE transpose). Known dead ends: multi-queue
gathers without dedicated sems (intermittent corruption), >1 group of
untriggered preps (deadlocks the 1024-desc carveout), single_packet=True
(wedges the device), bigger carveout (no gain - drain is bandwidth-bound).
"""

import math
from contextlib import ExitStack

import numpy as np

import concourse.bacc as bacc
import concourse.bass as bass
import concourse.tile as tile
from concourse import library_config, mybir
from concourse.bass_utils import run_bass_kernel_spmd

P = 128
D = 128
CORES = 8
N_NODES = 50000
N_EDGES = 800000

AF = mybir.ActivationFunctionType
OP = mybir.AluOpType
dt = mybir.dt


def _plan(n_nodes, cores):
    nloc = n_nodes // cores
    assert nloc * cores == n_nodes
    nb = math.ceil(nloc / P)
    nloc_pad = nb * P
    npad = cores * nloc_pad
    # lo/hi table halves split each core's rows at nlh so each half can be
    # all-gathered (and consumed) independently: table_lo[c*nlh + r] etc.
    nlh = nloc_pad // 2
    assert cores * nlh < 32768, "dma_gather idx is int16"
    return nloc, nb, nloc_pad, npad, nlh


def preprocess(edge_index, n_nodes, cores):
    """Returns per-core gather/scatter metadata + uniform chunk counts K0, K1."""
    nloc, nb, nloc_pad, npad, nlh = _plan(n_nodes, cores)
    src = np.asarray(edge_index[0], dtype=np.int64)
    dst = np.asarray(edge_index[1], dtype=np.int64)
    E = src.shape[0]

    deg = np.bincount(dst, minlength=n_nodes).astype(np.float64)
    invdeg = (1.0 / np.maximum(deg, 1.0)).astype(np.float32)

    csrc = src // nloc
    lsrc = src - csrc * nloc  # local (padded) row id of source within core
    half = (lsrc >= nlh).astype(np.int64)
    idx_in_half = (csrc * nlh + lsrc - half * nlh).astype(np.int64)

    cdst = dst // nloc
    ldst = dst - cdst * nloc
    blk = ldst // P
    dblk = ldst % P

    # sort edges by (dst core, dst block, src half, src row) — src order gives
    # the DMA engines ascending-address locality within each gather list
    order = np.lexsort((idx_in_half, half, blk, cdst))
    s_half = half[order]
    s_idx = idx_in_half[order]
    s_dblk = dblk[order]
    key = ((cdst[order] * nb + blk[order]) * 2 + s_half).astype(np.int64)

    counts = np.bincount(key, minlength=cores * nb * 2)
    starts = np.zeros(cores * nb * 2 + 1, dtype=np.int64)
    np.cumsum(counts, out=starts[1:])
    rank = np.arange(E, dtype=np.int64) - starts[key]

    cnt = counts.reshape(cores, nb, 2)
    K0 = max(1, int(math.ceil(cnt[:, :, 0].max() / P)))
    K1 = max(1, int(math.ceil(cnt[:, :, 1].max() / P)))

    # idx arrays: [cores, nb, K*P] int16 (pad = 0, harmless row gathered,
    # neutralized by dloc pad = 255 in the one-hot); dloc: [cores, nb, (K0+K1)*P]
    idx0 = np.zeros((cores, nb, K0 * P), dtype=np.int16)
    idx1 = np.zeros((cores, nb, K1 * P), dtype=np.int16)
    dloc = np.full((cores, nb, (K0 + K1) * P), 255.0, dtype=np.float16)

    core_k = key // (nb * 2)
    blk_k = (key // 2) % nb
    m0 = s_half == 0
    m1 = ~m0
    idx0[core_k[m0], blk_k[m0], rank[m0]] = s_idx[m0].astype(np.int16)
    idx1[core_k[m1], blk_k[m1], rank[m1]] = s_idx[m1].astype(np.int16)
    dloc[core_k[m0], blk_k[m0], rank[m0]] = s_dblk[m0].astype(np.float16)
    dloc[core_k[m1], blk_k[m1], K0 * P + rank[m1]] = s_dblk[m1].astype(np.float16)

    def wrap_idx(a):  # [nb, K*P] -> [128, nb*K*P//16] dma_gather layout
        flat = a.reshape(-1)
        w = flat.reshape(-1, 16).T  # [16, I/16]
        return np.tile(w, (8, 1)).copy()

    per_core = []
    for c in range(cores):
        dl = dloc[c].reshape(nb, K0 + K1, P).transpose(2, 0, 1).reshape(P, -1)
        inv = np.ones(nloc_pad, dtype=np.float32)
        inv[:nloc] = invdeg[c * nloc : (c + 1) * nloc]
        per_core.append(
            dict(
                idx0=wrap_idx(idx0[c]),
                idx1=wrap_idx(idx1[c]),
                dloc=np.ascontiguousarray(dl),
                invd=np.broadcast_to(inv[None, :], (P, nloc_pad)).copy(),
            )
        )
    return per_core, K0, K1, invdeg


NQ = 4  # SWDGE queues: each maps to its own gpsimd core pair -> 4x desc-gen
NSPLIT = 4  # sub-gathers per (group, half), round-robined across queues


def _check_lane_queue_alignment(nc):
    """Tile assigns SWDGE completions to 8 DMASW sem lanes round-robin in
    scheduled order; correctness of cross-queue gathers requires each lane to
    only ever carry one hardware queue (per-queue rings are FIFO, lanes count
    cumulatively). Verify the alignment held after scheduling."""
    lane_queues = {}
    for inst in nc.inst_map.values():
        if type(inst).__name__ != "InstDMAGatherAnt":
            continue
        if getattr(inst, "gen_mode", 0) == 1:
            continue  # prepare_only gathers use explicit gsem waits
        si = getattr(inst, "sync_info", None)
        assert si is not None
        for u in si.on_update:
            name = getattr(u, "ant_name", "") or ""
            if name.startswith("DMASW"):
                lane = name.split("_")[0]
                lane_queues.setdefault(lane, set()).add(inst.queue_num)
    for lane, qs in lane_queues.items():
        assert len(qs) == 1, f"sem lane {lane} carries queues {qs}: unsafe"


def build_nc(n_nodes, cores, K0, K1, G, iters=1):
    nloc, nb, nloc_pad, npad, nlh = _plan(n_nodes, cores)
    assert nb % G == 0
    ngroups = nb // G
    KT = K0 + K1
    blo = nlh // P + 1  # proj block count covering the lo half rows

    nc = bacc.Bacc(
        "TRN2",
        target_bir_lowering=False,
        debug=False,
        num_devices=cores,
        num_swdge_queues=NQ,
    )

    xT_d = nc.dram_tensor("xT", [P, nloc_pad], dt.float16, kind="ExternalInput").ap()
    idx0_d = nc.dram_tensor("idx0", [P, nb * K0 * P // 16], dt.int16, kind="ExternalInput").ap()
    idx1_d = nc.dram_tensor("idx1", [P, nb * K1 * P // 16], dt.int16, kind="ExternalInput").ap()
    dloc_d = nc.dram_tensor("dloc", [P, nb * KT], dt.float16, kind="ExternalInput").ap()
    invd_d = nc.dram_tensor("invd", [P, nloc_pad], dt.float32, kind="ExternalInput").ap()
    wdram = {
        n: nc.dram_tensor(n, [P, D], dt.float16, kind="ExternalInput").ap()
        for n in ["Wp1T", "Wl1T", "Wr1T", "Wp2T", "Wl2T", "Wr2T"]
    }
    bp1b_d = nc.dram_tensor("bp1b", [P, D], dt.float32, kind="ExternalInput").ap()
    bl1c_d = nc.dram_tensor("bl1c", [P, 1], dt.float32, kind="ExternalInput").ap()
    bp2b_d = nc.dram_tensor("bp2b", [P, D], dt.float32, kind="ExternalInput").ap()
    bl2b_d = nc.dram_tensor("bl2b", [P, D], dt.float32, kind="ExternalInput").ap()
    iota_d = nc.dram_tensor("iotaw", [P, KT * P], dt.float16, kind="ExternalInput").ap()

    out_own = nc.dram_tensor("out_own", [nloc_pad, D], dt.float32, kind="ExternalOutput").ap()
    h1own = nc.dram_tensor("h1own", [nloc_pad, D], dt.float16).ap()
    h2own = nc.dram_tensor("h2own", [nloc_pad, D], dt.float16).ap()
    t1lo = nc.dram_tensor("t1lo", [cores * nlh, D], dt.float16, addr_space="Shared").ap()
    t1hi = nc.dram_tensor("t1hi", [cores * nlh, D], dt.float16, addr_space="Shared").ap()
    t2lo = nc.dram_tensor("t2lo", [cores * nlh, D], dt.float16, addr_space="Shared").ap()
    t2hi = nc.dram_tensor("t2hi", [cores * nlh, D], dt.float16, addr_space="Shared").ap()
    dumi = nc.dram_tensor("dumi", [16, D], dt.float16).ap()
    dumo = nc.dram_tensor("dumo", [cores * 16, D], dt.float16, addr_space="Shared").ap()

    groups_all = [list(range(cores))]

    with tile.TileContext(nc) as tc, ExitStack() as ctx:
        const = ctx.enter_context(tc.tile_pool(name="const", bufs=1))
        persist = ctx.enter_context(tc.tile_pool(name="persist", bufs=1))
        stage_p = ctx.enter_context(tc.tile_pool(name="stage", bufs=3))
        work = ctx.enter_context(tc.tile_pool(name="work", bufs=3))
        ohp = ctx.enter_context(tc.tile_pool(name="oh", bufs=8))
        aggsb = ctx.enter_context(tc.tile_pool(name="aggsb", bufs=2))
        outp = ctx.enter_context(tc.tile_pool(name="outp", bufs=3))
        psum_agg = ctx.enter_context(tc.tile_pool(name="psum_agg", bufs=4, space="PSUM"))
        psum_mm = ctx.enter_context(tc.tile_pool(name="psum_mm", bufs=2, space="PSUM"))

        nc.gpsimd.load_library(library_config.mlp)

        def cload(ap_dram, shape, dtype, tag):
            t = const.tile(shape, dtype, tag=tag)
            nc.sync.dma_start(t[:], ap_dram)
            return t

        wsb = {n: cload(wdram[n][:, :], [P, D], dt.float16, n) for n in wdram}
        bp1b = cload(bp1b_d[:, :], [P, D], dt.float32, "bp1b")
        bl1c = cload(bl1c_d[:, :], [P, 1], dt.float32, "bl1c")
        bp2b = cload(bp2b_d[:, :], [P, D], dt.float32, "bp2b")
        bl2b = cload(bl2b_d[:, :], [P, D], dt.float32, "bl2b")
        iota_w = cload(iota_d[:, :], [P, KT * P], dt.float16, "iotaw")
        dloc_sb = cload(dloc_d[:, :], [P, nb * KT], dt.float16, "dloc")
        invd_sb = cload(invd_d[:, :], [P, nloc_pad], dt.float32, "invd")
        idx0_sb = cload(idx0_d[:, :], [P, nb * K0 * P // 16], dt.int16, "idx0")
        idx1_sb = cload(idx1_d[:, :], [P, nb * K1 * P // 16], dt.int16, "idx1")

        xT_sb = persist.tile([P, nloc_pad], dt.float16, tag="xT")
        h1T_sb = persist.tile([P, nloc_pad], dt.float16, tag="h1T")

        # Gather-completion semaphores: one per (queue, half, group-parity).
        # sem= on a prepare_only gather is baked into its descriptors; the
        # consumer waits for the round's total (16 incs per sub-gather), an
        # order-insensitive condition. Same-sem rounds are SPAR groups apart
        # and SPAR == staging bufs, so the staging-tile WAR serializes them —
        # cumulative targets can never be satisfied early by a later round.
        SPAR = 3  # must equal stage_p bufs
        gsem = [
            [[nc.alloc_semaphore(f"gs{q}_{h}_{p}") for p in range(SPAR)] for h in range(2)]
            for q in range(NQ)
        ]

        def _iter_body():
            for qs in gsem:
                for hs in qs:
                    for s in hs:
                        nc.gpsimd.sem_clear(s)
            gtgt = [[[0] * SPAR for _ in range(2)] for _ in range(NQ)]
            grp = [0]  # global group counter: parity follows pool rotation

            # dummy collective: absorbs the one-time comm-init barrier while
            # the projection phase runs
            nc.gpsimd.collective_compute(
                "AllGather", OP.bypass, replica_groups=groups_all,
                ins=[dumi[:, :]], outs=[dumo[:, :]],
            )

            def proj2_block(b):
                sl = slice(b * P, (b + 1) * P)
                p_ps = psum_mm.tile([P, D], dt.float32, tag="mm")
                nc.tensor.matmul(p_ps[:], lhsT=h1T_sb[:, sl], rhs=wsb["Wp2T"][:], start=True, stop=True)
                pb = work.tile([P, D], dt.float32, tag="pb")
                nc.vector.tensor_tensor(out=pb[:], in0=p_ps[:], in1=bp2b[:], op=OP.add)
                pr = outp.tile([P, D], dt.float16, tag="pr")
                nc.scalar.activation(pr[:], pb[:], AF.Relu)
                nc.sync.dma_start(h2own[sl, :], pr[:])

            def ag(h_own, tlo, thi, part):
                nc.gpsimd.collective_compute(
                    "AllGather", OP.bypass, replica_groups=groups_all,
                    ins=[h_own[0:nlh, :] if part == 0 else h_own[nlh:nloc_pad, :]],
                    outs=[(tlo if part == 0 else thi)[:, :]],
                )

            def split_ranges(n):
                step = math.ceil(n / NSPLIT)
                return [(s, min(s + step, n)) for s in range(0, n, step)]

            # ---------------- message+aggregate for one layer -------------------
            # prepare_only sub-gathers fan descriptor generation across the 4
            # SWDGE queue core-pairs and don't wait for table data (only the
            # triggers carry that dep, so desc-gen hides under the AllGather).
            # One prep per queue per trigger round keeps the untriggered ring
            # footprint at the same level as a regular triggered gather.
            def prep_group(staged, g, tlo, thi, trigger=True):
                par = grp[0] % SPAR
                grp[0] += 1
                st0 = stage_p.tile([P, G * K0, D], dt.float16, tag="st0")
                st1 = stage_p.tile([P, G * K1, D], dt.float16, tag="st1")
                staged[g] = (st0, st1, par)
                for h, (st, idx_sb, K) in enumerate((
                    (st0, idx0_sb, K0),
                    (st1, idx1_sb, K1),
                )):
                    tab = tlo if h == 0 else thi
                    gcols = g * G * K * P // 16
                    for j, (s, e) in enumerate(split_ranges(G * K)):
                        q = j % NQ
                        nc.gpsimd.dma_gather(
                            st[:, s:e, :],
                            tab[:, :],
                            idx_sb[:, gcols + s * 8 : gcols + e * 8],
                            (e - s) * P, (e - s) * P, D,
                            single_packet=False,
                            queue_num=q,
                            prepare_only=True,
                            sem=gsem[q][h][par],
                        )
                        gtgt[q][h][par] += 16
                    if trigger:
                        for q in range(NQ):
                            nc.gpsimd.trigger_dma(count=None, queue_num=q)

            # ---------------- Phase A: layer-1 projection of own rows ----------
            nc.sync.dma_start(xT_sb[:], xT_d[:, :])
            for b in range(nb):
                sl = slice(b * P, (b + 1) * P)
                p_ps = psum_mm.tile([P, D], dt.float32, tag="mm")
                nc.tensor.matmul(p_ps[:], lhsT=xT_sb[:, sl], rhs=wsb["Wp1T"][:], start=True, stop=True)
                pb = work.tile([P, D], dt.float32, tag="pb")
                nc.vector.tensor_tensor(out=pb[:], in0=p_ps[:], in1=bp1b[:], op=OP.add)
                pr = outp.tile([P, D], dt.float16, tag="pr")
                nc.scalar.activation(pr[:], pb[:], AF.Relu)
                nc.sync.dma_start(h1own[sl, :], pr[:])
                if b == blo - 1:
                    ag(h1own, t1lo, t1hi, 0)
            ag(h1own, t1lo, t1hi, 1)

            def agg_layer(tlo, thi, root_sb, WlT, WrT, layer, staged):
                for g in range(ngroups):
                    if g not in staged:
                        prep_group(staged, g, tlo, thi)
                    else:
                        # pre-prepped before Phase A: fire its deferred triggers
                        for q in range(NQ):
                            nc.gpsimd.trigger_dma(count=None, queue_num=q)
                    st0, st1, par = staged.pop(g)
                    first = True
                    for bb in range(G):
                        b = g * G + bb
                        sl = slice(b * P, (b + 1) * P)
                        ohb = ohp.tile([P, KT, P], dt.float16)
                        nc.vector.tensor_tensor(
                            out=ohb[:],
                            in0=dloc_sb[:, b * KT : (b + 1) * KT].to_broadcast([P, KT, P]),
                            in1=iota_w[:, :].rearrange("p (k d) -> p k d", k=KT),
                            op=OP.is_equal,
                        )
                        if first:
                            for q in range(NQ):
                                for h in range(2):
                                    nc.tensor.wait_ge(gsem[q][h][par], gtgt[q][h][par])
                            first = False
                        agg_ps = psum_agg.tile([P, P], dt.float32)
                        for t in range(KT):
                            msg = st0[:, bb * K0 + t, :] if t < K0 else st1[:, bb * K1 + (t - K0), :]
                            nc.tensor.matmul(
                                agg_ps[:], lhsT=msg, rhs=ohb[:, t, :],
                                start=(t == 0), stop=(t == KT - 1),
                            )
                        aggT = aggsb.tile([P, P], dt.float16)
                        nc.vector.tensor_tensor(
                            out=aggT[:], in0=agg_ps[:], in1=invd_sb[:, sl], op=OP.mult
                        )
                        if layer == 1:
                            o_ps = psum_mm.tile([P, P], dt.float32, tag="mm")
                            nc.tensor.matmul(o_ps[:], lhsT=WlT[:], rhs=aggT[:], start=True, stop=False)
                            nc.tensor.matmul(o_ps[:], lhsT=WrT[:], rhs=root_sb[:, sl], start=False, stop=True)
                            nc.scalar.activation(h1T_sb[:, sl], o_ps[:], AF.Relu, bias=bl1c[:], scale=1.0)
                            proj2_block(b)
                            if b == blo - 1:
                                ag(h2own, t2lo, t2hi, 0)
                            elif b == nb - 1:
                                ag(h2own, t2lo, t2hi, 1)
                        else:
                            o_ps = psum_mm.tile([P, D], dt.float32, tag="mm")
                            nc.tensor.matmul(o_ps[:], lhsT=aggT[:], rhs=WlT[:], start=True, stop=False)
                            nc.tensor.matmul(o_ps[:], lhsT=root_sb[:, sl], rhs=WrT[:], start=False, stop=True)
                            ob = outp.tile([P, D], dt.float32, tag="ob")
                            nc.vector.tensor_tensor(out=ob[:], in0=o_ps[:], in1=bl2b[:], op=OP.add)
                            nc.sync.dma_start(out_own[sl, :], ob[:])

            # ---------------- Phase B: layer-1 aggregate (+proj2, AG2) ---------
            agg_layer(t1lo, t1hi, xT_sb, wsb["Wl1T"], wsb["Wr1T"], layer=1, staged={})

            # ---------------- Phase D: layer-2 aggregate -> out ----------------
            agg_layer(t2lo, t2hi, h1T_sb, wsb["Wl2T"], wsb["Wr2T"], layer=2, staged={})

        for _ in range(iters):
            _iter_body()

    nc.compile()
    _check_lane_queue_alignment(nc)
    return nc


def make_in_maps(inputs, per_core, n_nodes, cores, KT):
    nloc, nb, nloc_pad, npad, nhalf = _plan(n_nodes, cores)
    x = np.asarray(inputs["x"], dtype=np.float32)
    consts = dict(
        Wp1T=np.asarray(inputs["Wp1"]).T.astype(np.float16),
        Wl1T=np.asarray(inputs["Wl1"]).T.astype(np.float16),
        Wr1T=np.asarray(inputs["Wr1"]).T.astype(np.float16),
        Wp2T=np.asarray(inputs["Wp2"]).T.astype(np.float16),
        Wl2T=np.asarray(inputs["Wl2"]).T.astype(np.float16),
        Wr2T=np.asarray(inputs["Wr2"]).T.astype(np.float16),
        bp1b=np.broadcast_to(np.asarray(inputs["bp1"], np.float32)[None, :], (P, D)).copy(),
        bl1c=np.asarray(inputs["bl1"], np.float32).reshape(P, 1).copy(),
        bp2b=np.broadcast_to(np.asarray(inputs["bp2"], np.float32)[None, :], (P, D)).copy(),
        bl2b=np.broadcast_to(np.asarray(inputs["bl2"], np.float32)[None, :], (P, D)).copy(),
        iotaw=np.broadcast_to(
            np.tile(np.arange(P, dtype=np.float16), KT)[None, :], (P, KT * P)
        ).copy(),
    )
    in_maps = []
    for c in range(cores):
        xo = np.zeros((nloc_pad, D), dtype=np.float32)
        xo[:nloc] = x[c * nloc : (c + 1) * nloc]
        m = dict(consts)
        m["xT"] = np.ascontiguousarray(xo.T.astype(np.float16))
        m.update(per_core[c])
        in_maps.append(m)
    return in_maps


_BUILT = {}


def _run(inputs, n_nodes, n_edges, cores, G, trace=False):
    per_core, K0, K1, _ = preprocess(inputs["edge_index"], n_nodes, cores)
    key = (n_nodes, cores, K0, K1, G)
    if key not in _BUILT:
        _BUILT[key] = build_nc(n_nodes, cores, K0, K1, G)
    nc = _BUILT[key]
    in_maps = make_in_maps(inputs, per_core, n_nodes, cores, K0 + K1)
    res = run_bass_kernel_spmd(nc, in_maps, list(range(cores)), trace=trace)
    nloc, nb, nloc_pad, npad, nhalf = _plan(n_nodes, cores)
    out = np.concatenate([res.results[c]["out_own"][:nloc] for c in range(cores)], axis=0)
    return out.astype(np.float32), res


def kernel(**inputs):
    out, _ = _run(inputs, N_NODES, N_EDGES, CORES, G=7)
    return out

